# revision 29
# baseline (speedup 1.0000x reference)
import sys

sys.path.insert(0, "/opt/trn_rl_repo")

import numpy as np
import ml_dtypes

from concourse import bass, mybir
from concourse.tile import TileContext
from concourse.bass_utils import run_bass_kernel_spmd

dt = mybir.dt
Alu = mybir.AluOpType
Act = mybir.ActivationFunctionType

H = 4096
W = 4096
NCORES = 8
RPC = H // NCORES            # 512 output rows per core
HALO = 4                     # blur(2) + sobel(1) + nms(1)
SH = RPC + 2 * HALO          # 520 input rows per core
BASES = (0, 120, 240, 360, 392)
NT = len(BASES)
NCH = 3
CW = 512                     # matmul chunk (one PSUM bank)
SCW = 1024                   # superchunk for elementwise stages
NSC = W // SCW
P = 128
BF16 = ml_dtypes.bfloat16

TAN_LO = float(np.float32(np.tan(3.14159 / 8)))
TAN_HI = float(np.float32(np.tan(3 * 3.14159 / 8)))
TAN_LO2 = TAN_LO * TAN_LO
TAN_HI2 = TAN_HI * TAN_HI
LOWER_T = 6.0
UPPER_T = 50.0

# wb column layout: 5 blur bands, then per tile-variant {mid, t0, t4} the
# six sobel/shift bands [V121, NV121, U, U2, SU, SD]
GO_MID = 5 * P
GO_T0 = GO_MID + 6 * P
GO_T4 = GO_T0 + 6 * P
WBW = GO_T4 + 6 * P          # 2944


def _band(taps, r):
    # lhsT[k, m] = taps[k - m + r]  => out[m] = sum_k taps[k-m+r] * x[k]
    L = np.zeros((P, P), np.float32)
    for i, tv in enumerate(taps):
        L += np.float32(tv) * np.eye(P, k=r - i, dtype=np.float32)
    return L


def _weights(gauss, is_top, is_bot):
    g = np.asarray(gauss, np.float32)
    bg = _band(g, 2)
    v121 = _band([1.0, 2.0, 1.0], 1)
    u = _band([1.0, 0.0, -1.0], 1)
    su = _band([1.0], 1)
    sd = _band([1.0], -1)

    def group(zero_row, zero_su, zero_sd):
        mats = [v121.copy(), -v121, u.copy(), 2.0 * u, su.copy(), sd.copy()]
        if zero_row is not None:
            for idx in (0, 1, 2, 3):
                mats[idx][zero_row, :] = 0.0
            if zero_su:
                mats[4][zero_row, :] = 0.0
            if zero_sd:
                mats[5][zero_row, :] = 0.0
        return mats

    cols = [bg * g[d] for d in range(5)]
    cols += group(None, False, False)                       # mid
    cols += group(3 if is_top else None, True, False)       # t0 variant
    cols += group(124 if is_bot else None, False, True)     # t4 variant
    wb = np.concatenate(cols, axis=1)
    assert wb.shape == (P, WBW)
    return wb.astype(BF16)


# per-site engine assignment ('v'=DVE, 'a'=ACT for PSUM drains; 'g'=GpSimd is
# only legal for plain TT add/sub/mult + tensor_scalar, and on real HW costs
# ~2.6us per 1024-wide op (~3x DVE), so use it sparingly).
# Multi-char values alternate by superchunk index for fractional balance.
DEFAULT_ENG = {
    'blurcp': 'a',            # blur PSUM drain [128,1024]: 'v' or 'a'
    'gxycp': 'a',             # fused gx|gy PSUM drain [128,2048]: 'v' or 'a'
    'gucp': 'a', 'gdcp': 'a', # SU/SD shift drains [128,512]
    'sq': 'g',                # squares of gxyb (TT mult, 2048): 'v' or 'g'
    'ss': 'gv',                # sqx+sqy add (1024): 'v' or 'g'
    'acc': 'v',               # channel accumulate (1024): 'v' or 'g'
    'sums': 'g',              # channel sums of gxyb (2048): 'v' or 'g'
    'gg2': 'v',               # square of gsB (2048): 'v' or 'g'
    'thtl': 'v',              # TS scalings of gxs^2 (DVE; GpSimd TS is ~15us!)
    'c0c2': 'v',              # c0M/c2M u16 compares (DVE only)
    'sxy': 'v',               # sign product (TT mult): 'v' or 'g'
    'csM': 'v',               # sign mask (TS, DVE)
    'm1': 'v', 'msel': 'v', 'm0': 'v', 'm2u': 'v',   # TT max: DVE only
    'thr': 'v',               # STT: DVE only
}


def _build_nc(reps=1, eng=None):
    eng = dict(DEFAULT_ENG, **(eng or {}))
    nc = bass.Bass(trn_type="TRN2")
    x_d = nc.dram_tensor("x", (NCH, SH, W + 4), dt.bfloat16, kind="ExternalInput")
    wb_d = nc.dram_tensor("wb", (P, WBW), dt.bfloat16, kind="ExternalInput")
    out_d = nc.dram_tensor("out", (RPC, W), dt.bfloat16, kind="ExternalOutput")

    def EV(key, sc=0):
        e = eng[key]
        e = e[sc % len(e)]
        assert e in ('v', 'g')
        return nc.gpsimd if e == 'g' else nc.vector

    def drain(key, sc, out_ap, ps_ap):
        e = eng[key]
        e = e[sc % len(e)]
        if e == 'a':
            nc.scalar.activation(out_ap, ps_ap, Act.Copy)
        else:
            nc.vector.tensor_copy(out_ap, ps_ap)

    with TileContext(nc) as tc:
        with tc.tile_pool(name="sb", bufs=2) as pool, \
             tc.tile_pool(name="ps", bufs=2, space="PSUM") as pp:
            wb_sb = pool.tile([P, WBW], dt.bfloat16, tag="wb", bufs=1)
            nc.sync.dma_start(wb_sb[:, :], wb_d[:, :])

            # reps>1 exists only for benchmarking (test.py): a hardware loop
            # repeats the identical full pipeline on-device so one dispatch
            # amortizes the host/tunnel launch overhead over `reps` runs
            _rep = tc.For_i(0, reps, 1, hint_engines=(mybir.EngineType.PE, mybir.EngineType.DVE, mybir.EngineType.Activation, mybir.EngineType.SP), staggered_reset=True) if reps > 1 else None
            if _rep is not None:
                _rep.__enter__()
            for t in range(NT):
                base = BASES[t]
                go = GO_T0 if t == 0 else (GO_T4 if t == NT - 1 else GO_MID)
                V121 = wb_sb[:, go:go + P]
                NV121 = wb_sb[:, go + P:go + 2 * P]
                U = wb_sb[:, go + 2 * P:go + 3 * P]
                U2 = wb_sb[:, go + 3 * P:go + 4 * P]
                SU = wb_sb[:, go + 4 * P:go + 5 * P]
                SD = wb_sb[:, go + 5 * P:go + 6 * P]

                xs, bls = [], []
                for c in range(NCH):
                    x_sb = pool.tile([P, W + 4], dt.bfloat16, tag=f"x{c}", bufs=1)
                    nc.sync.dma_start(x_sb[:, :], x_d[c, base:base + P, :])
                    xs.append(x_sb)
                    bl = pool.tile([P, W + 2], dt.bfloat16, tag=f"blur{c}", bufs=2)
                    nc.vector.memset(bl[:, 0:1], 0.0)
                    nc.vector.memset(bl[:, W + 1:W + 2], 0.0)
                    bls.append(bl)

                grad = pool.tile([P, W + 2], dt.bfloat16, tag="grad", bufs=2)
                nc.vector.memset(grad[:, 0:1], 0.0)
                nc.vector.memset(grad[:, W + 1:W + 2], 0.0)
                gU = pool.tile([P, W + 2], dt.bfloat16, tag="gU", bufs=2)
                gD = pool.tile([P, W + 2], dt.bfloat16, tag="gD", bufs=2)
                obF = pool.tile([P, W], dt.bfloat16, tag="obF", bufs=1)

                def blur_sc(sc, c):
                    # 5x5 blur for superchunk sc, channel c
                    ps = pp.tile([P, SCW], dt.float32, tag="psb", bufs=1)
                    for half in range(2):
                        j0 = sc * SCW + half * CW
                        for d in range(5):
                            nc.tensor.matmul(
                                out=ps[:, half * CW:half * CW + CW],
                                lhsT=wb_sb[:, d * P:(d + 1) * P],
                                rhs=xs[c][:, j0 + d:j0 + d + CW],
                                start=(d == 0), stop=(d == 4),
                            )
                    drain('blurcp', sc, bls[c][:, 1 + sc * SCW:1 + (sc + 1) * SCW], ps[:, :])

                gxybs = [None] * NCH

                def stage_b_ch(sc, c):
                    # sobel + magnitude for superchunk sc, channel c
                    bl = bls[c]
                    pgxy = pp.tile([P, 2 * SCW], dt.float32, tag="pgxy", bufs=1)
                    for half in range(2):
                        pj = 1 + sc * SCW + half * CW
                        hx = slice(half * CW, half * CW + CW)
                        hy = slice(SCW + half * CW, SCW + half * CW + CW)
                        nc.tensor.matmul(out=pgxy[:, hx], lhsT=V121,
                                         rhs=bl[:, pj - 1:pj - 1 + CW], start=True, stop=False)
                        nc.tensor.matmul(out=pgxy[:, hx], lhsT=NV121,
                                         rhs=bl[:, pj + 1:pj + 1 + CW], start=False, stop=True)
                        nc.tensor.matmul(out=pgxy[:, hy], lhsT=U,
                                         rhs=bl[:, pj - 1:pj - 1 + CW], start=True, stop=False)
                        nc.tensor.matmul(out=pgxy[:, hy], lhsT=U2,
                                         rhs=bl[:, pj:pj + CW], start=False, stop=False)
                        nc.tensor.matmul(out=pgxy[:, hy], lhsT=U,
                                         rhs=bl[:, pj + 1:pj + 1 + CW], start=False, stop=True)
                    gxyb = pool.tile([P, 2 * SCW], dt.bfloat16, tag="gxyb", bufs=3)
                    drain('gxycp', sc, gxyb[:, :], pgxy[:, :])
                    gxybs[c] = gxyb
                    sqf = pool.tile([P, 2 * SCW], dt.bfloat16, tag="sqf", bufs=2)
                    sqe = eng['sq'][sc % len(eng['sq'])]
                    if sqe == 'a':
                        # square straight off PSUM on ACT (second reader of pgxy)
                        nc.scalar.activation(sqf[:, :], pgxy[:, :], Act.Square)
                    else:
                        ev = nc.gpsimd if sqe == 'g' else nc.vector
                        ev.tensor_tensor(sqf[:, 0:SCW], gxyb[:, 0:SCW],
                                         gxyb[:, 0:SCW], Alu.mult)
                        ev.tensor_tensor(sqf[:, SCW:2 * SCW], gxyb[:, SCW:2 * SCW],
                                         gxyb[:, SCW:2 * SCW], Alu.mult)
                    # ss = sqx + sqy, in place over the sqx half
                    EV('ss', sc).tensor_tensor(sqf[:, 0:SCW], sqf[:, 0:SCW],
                                               sqf[:, SCW:2 * SCW], Alu.add)
                    pj = 1 + sc * SCW
                    if c == 0:
                        nc.scalar.activation(grad[:, pj:pj + SCW], sqf[:, 0:SCW], Act.Sqrt)
                    else:
                        # magnitude into the dead sqy half, then accumulate
                        nc.scalar.activation(sqf[:, SCW:2 * SCW], sqf[:, 0:SCW], Act.Sqrt)
                        EV('acc', sc).tensor_tensor(grad[:, pj:pj + SCW], grad[:, pj:pj + SCW],
                                                    sqf[:, SCW:2 * SCW], Alu.add)

                def shifts(sc, half):
                    # rolling production of the +-1 row shifts of grad via
                    # SU/SD band matmuls; window [g0+2, g0+1025] in padded
                    # cols (plus a 2-col top-up at the tile start) so that
                    # stage_c(sc) reads cols [g0, g0+1025] fully covered
                    pU = pp.tile([P, CW], dt.float32, tag="pU", bufs=1)
                    pD = pp.tile([P, CW], dt.float32, tag="pD", bufs=1)
                    c0 = sc * SCW + 2 + half * CW
                    nc.tensor.matmul(out=pU[:, :], lhsT=SU, rhs=grad[:, c0:c0 + CW],
                                     start=True, stop=True)
                    nc.tensor.matmul(out=pD[:, :], lhsT=SD, rhs=grad[:, c0:c0 + CW],
                                     start=True, stop=True)
                    drain('gucp', sc, gU[:, c0:c0 + CW], pU[:, :])
                    drain('gdcp', sc, gD[:, c0:c0 + CW], pD[:, :])
                    if sc == 0 and half == 0:
                        pU2 = pp.tile([P, CW], dt.float32, tag="pU", bufs=1)
                        pD2 = pp.tile([P, CW], dt.float32, tag="pD", bufs=1)
                        nc.tensor.matmul(out=pU2[:, 0:2], lhsT=SU, rhs=grad[:, 0:2],
                                         start=True, stop=True)
                        nc.tensor.matmul(out=pD2[:, 0:2], lhsT=SD, rhs=grad[:, 0:2],
                                         start=True, stop=True)
                        drain('gucp', sc, gU[:, 0:2], pU2[:, 0:2])
                        drain('gdcp', sc, gD[:, 0:2], pD2[:, 0:2])

                def masks_sc(sc):
                    # channel sums + orientation masks for superchunk sc;
                    # also precompute the (grad <= 50) factor here so the
                    # latency-critical stage_c chain gets one op shorter
                    b50 = pool.tile([P, SCW], dt.bfloat16, tag="b50", bufs=2)
                    pj50 = 1 + sc * SCW
                    nc.vector.tensor_scalar(b50[:, :], grad[:, pj50:pj50 + SCW], UPPER_T,
                                            None, Alu.is_le)
                    gsB = pool.tile([P, 2 * SCW], dt.bfloat16, tag="gsB", bufs=2)
                    for hs in (slice(0, SCW), slice(SCW, 2 * SCW)):
                        EV('sums', sc).tensor_tensor(gsB[:, hs], gxybs[0][:, hs],
                                                     gxybs[1][:, hs], Alu.add)
                        EV('sums', sc).tensor_tensor(gsB[:, hs], gsB[:, hs],
                                                     gxybs[2][:, hs], Alu.add)
                    sxy = pool.tile([P, SCW], dt.bfloat16, tag="sxy", bufs=2)
                    EV('sxy', sc).tensor_tensor(sxy[:, :], gsB[:, 0:SCW], gsB[:, SCW:2 * SCW],
                                                Alu.mult)
                    csM = pool.tile([P, SCW], dt.uint16, tag="csM", bufs=2)
                    nc.vector.tensor_scalar(csM[:, :], sxy[:, :], 0.0, None, Alu.is_gt)
                    # square the sums in place: gsB -> [gxs^2 | gys^2]
                    EV('gg2', sc).tensor_tensor(gsB[:, 0:SCW], gsB[:, 0:SCW],
                                                gsB[:, 0:SCW], Alu.mult)
                    EV('gg2', sc).tensor_tensor(gsB[:, SCW:2 * SCW], gsB[:, SCW:2 * SCW],
                                                gsB[:, SCW:2 * SCW], Alu.mult)
                    # scaled copies of gxs^2 via cheap 4x-mode TS, then u16
                    # compares (cheaper than two 1x-mode STT ops)
                    th = pool.tile([P, SCW], dt.bfloat16, tag="th", bufs=2)
                    EV('thtl', sc).tensor_scalar(th[:, :], gsB[:, 0:SCW], TAN_HI2, None, Alu.mult)
                    c2M = pool.tile([P, SCW], dt.uint16, tag="c2M", bufs=2)
                    EV('c0c2', sc).tensor_tensor(c2M[:, :], th[:, :],
                                                 gsB[:, SCW:2 * SCW], Alu.is_lt)
                    tl = pool.tile([P, SCW], dt.bfloat16, tag="th", bufs=2)
                    EV('thtl', sc).tensor_scalar(tl[:, :], gsB[:, 0:SCW], TAN_LO2, None, Alu.mult)
                    c0M = pool.tile([P, SCW], dt.uint16, tag="c0M", bufs=2)
                    EV('c0c2', sc).tensor_tensor(c0M[:, :], tl[:, :],
                                                 gsB[:, SCW:2 * SCW], Alu.is_gt)
                    return (csM, c0M, c2M, b50)

                def stage_c(sc, masks):
                    csM, c0M, c2M, b50 = masks
                    g0 = sc * SCW
                    pj = 1 + g0
                    m1 = pool.tile([P, SCW], dt.bfloat16, tag="m1", bufs=2)
                    EV('m1', sc).tensor_tensor(m1[:, :], gD[:, pj + 1:pj + 1 + SCW],
                                               gU[:, pj - 1:pj - 1 + SCW], Alu.max)
                    msel = pool.tile([P, SCW], dt.bfloat16, tag="msel", bufs=2)
                    EV('msel', sc).tensor_tensor(msel[:, :], gD[:, pj - 1:pj - 1 + SCW],
                                                 gU[:, pj + 1:pj + 1 + SCW], Alu.max)
                    m0 = pool.tile([P, SCW], dt.bfloat16, tag="m0", bufs=2)
                    EV('m0', sc).tensor_tensor(m0[:, :], grad[:, pj - 1:pj - 1 + SCW],
                                               grad[:, pj + 1:pj + 1 + SCW], Alu.max)
                    m2u = pool.tile([P, SCW], dt.bfloat16, tag="m2u", bufs=2)
                    EV('m2u', sc).tensor_tensor(m2u[:, :], gU[:, pj:pj + SCW],
                                                gD[:, pj:pj + SCW], Alu.max)
                    nc.vector.copy_predicated(msel[:, :], csM[:, :], m1[:, :])
                    nc.vector.copy_predicated(msel[:, :], c0M[:, :], m0[:, :])
                    nc.vector.copy_predicated(msel[:, :], c2M[:, :], m2u[:, :])
                    # is_max & grad > 6:  max(msel, 6) < grad; the 6-clamp as
                    # a cheap in-place 4x-mode TS, then the compare (reuse m0)
                    nc.vector.tensor_scalar(msel[:, :], msel[:, :], LOWER_T, None, Alu.max)
                    EV('thr', sc).tensor_tensor(m0[:, :], msel[:, :],
                                                grad[:, pj:pj + SCW], Alu.is_lt)
                    # * precomputed (grad <= 50)
                    EV('thr', sc).tensor_tensor(obF[:, g0:g0 + SCW], b50[:, :],
                                                m0[:, :], Alu.mult)

                masks = [None] * NSC
                for it in range(NSC + 2):
                    for c in range(NCH):
                        if it < NSC:
                            blur_sc(it, c)
                        if 0 <= it - 1 < NSC:
                            stage_b_ch(it - 1, c)
                        if 0 <= it - 2 < NSC and c < 2:
                            shifts(it - 2, c)
                    if 0 <= it - 1 < NSC:
                        masks[it - 1] = masks_sc(it - 1)
                    if 0 <= it - 2 < NSC:
                        stage_c(it - 2, masks[it - 2])
                        masks[it - 2] = None

                # ---- stage D: store this tile's owned rows as bf16 0/1 ----
                if t < NT - 1:
                    nc.sync.dma_start(out_d[t * 120:t * 120 + 120, :], obF[4:124, :])
                else:
                    nc.sync.dma_start(out_d[480:512, :], obF[92:124, :])

            if _rep is not None:
                _rep.__exit__(None, None, None)

    import bass_rust
    # HW descriptors hold only one sync wait; park extras on Ldweights /
    # split the remainder into EventSemaphore instructions
    bass_rust.move_matmul_waits_to_ldweights(nc.m)
    bass_rust.generate_event_semaphores(nc)
    nc.finalize()
    return nc


def _shard_inputs(img, gauss):
    imgf = np.ascontiguousarray(img[0])  # [3, H, W] f32
    in_maps = []
    for k in range(NCORES):
        xk = np.zeros((NCH, SH, W + 4), dtype=BF16)
        lo = k * RPC - HALO
        hi = k * RPC + RPC + HALO
        s0, s1 = max(lo, 0), min(hi, H)
        xk[:, s0 - lo:s1 - lo, 2:W + 2] = imgf[:, s0:s1, :].astype(BF16)
        wbk = _weights(gauss, is_top=(k == 0), is_bot=(k == NCORES - 1))
        in_maps.append({"x": xk, "wb": wbk})
    return in_maps


def _assemble(results):
    full = np.empty((H, W), dtype=np.float32)
    for k in range(NCORES):
        ok = np.asarray(results[k]["out"])  # [512, 4096] bf16 0/1
        full[k * RPC:(k + 1) * RPC] = (ok != 0).astype(np.float32)
    return full.reshape(1, 1, H, W)


def _run(img, gauss, trace=False):
    nc = _build_nc()
    in_maps = _shard_inputs(np.asarray(img, np.float32), np.asarray(gauss, np.float32))
    res = run_bass_kernel_spmd(nc, in_maps, core_ids=list(range(NCORES)), trace=trace)
    return _assemble(res.results), res.exec_time_ns


def kernel(img=None, gauss=None, sobel=None, dir_w=None, **_):
    out, _ns = _run(img, gauss)
    return out


# revision 31
# speedup vs baseline: 1.2727x; 1.2727x over previous
import sys

sys.path.insert(0, "/opt/trn_rl_repo")

import numpy as np
import ml_dtypes

from concourse import bass, mybir
from concourse.tile import TileContext
from concourse.bass_utils import run_bass_kernel_spmd

dt = mybir.dt
Alu = mybir.AluOpType
Act = mybir.ActivationFunctionType

H = 4096
W = 4096
NCORES = 8
RPC = H // NCORES            # 512 output rows per core
HALO = 4                     # blur(2) + sobel(1) + nms(1)
SH = RPC + 2 * HALO          # 520 input rows per core
BASES = (0, 120, 240, 360, 392)
NT = len(BASES)
NCH = 3
CW = 512                     # matmul chunk (one PSUM bank)
SCW = 1024                   # superchunk for elementwise stages
NSC = W // SCW
P = 128
BF16 = ml_dtypes.bfloat16

TAN_LO = float(np.float32(np.tan(3.14159 / 8)))
TAN_HI = float(np.float32(np.tan(3 * 3.14159 / 8)))
TAN_LO2 = TAN_LO * TAN_LO
TAN_HI2 = TAN_HI * TAN_HI
LOWER_T = 6.0
UPPER_T = 50.0

# wb column layout: 5 blur bands, then per tile-variant {mid, t0, t4} the
# six sobel/shift bands [V121, NV121, U, U2, SU, SD]
GO_MID = 5 * P
GO_T0 = GO_MID + 6 * P
GO_T4 = GO_T0 + 6 * P
WBW = GO_T4 + 6 * P          # 2944


def _band(taps, r):
    # lhsT[k, m] = taps[k - m + r]  => out[m] = sum_k taps[k-m+r] * x[k]
    L = np.zeros((P, P), np.float32)
    for i, tv in enumerate(taps):
        L += np.float32(tv) * np.eye(P, k=r - i, dtype=np.float32)
    return L


def _weights(gauss, is_top, is_bot):
    g = np.asarray(gauss, np.float32)
    bg = _band(g, 2)
    v121 = _band([1.0, 2.0, 1.0], 1)
    u = _band([1.0, 0.0, -1.0], 1)
    su = _band([1.0], 1)
    sd = _band([1.0], -1)

    def group(zero_row, zero_su, zero_sd):
        mats = [v121.copy(), -v121, u.copy(), 2.0 * u, su.copy(), sd.copy()]
        if zero_row is not None:
            for idx in (0, 1, 2, 3):
                mats[idx][zero_row, :] = 0.0
            if zero_su:
                mats[4][zero_row, :] = 0.0
            if zero_sd:
                mats[5][zero_row, :] = 0.0
        return mats

    cols = [bg * g[d] for d in range(5)]
    cols += group(None, False, False)                       # mid
    cols += group(3 if is_top else None, True, False)       # t0 variant
    cols += group(124 if is_bot else None, False, True)     # t4 variant
    wb = np.concatenate(cols, axis=1)
    assert wb.shape == (P, WBW)
    return wb.astype(BF16)


# per-site engine assignment ('v'=DVE, 'a'=ACT for PSUM drains; 'g'=GpSimd is
# only legal for plain TT add/sub/mult + tensor_scalar, and on real HW costs
# ~2.6us per 1024-wide op (~3x DVE), so use it sparingly).
# Multi-char values alternate by superchunk index for fractional balance.
DEFAULT_ENG = {
    'blurcp': 'a',            # blur PSUM drain [128,1024]: 'v' or 'a'
    'gxycp': 'a',             # fused gx|gy PSUM drain [128,2048]: 'v' or 'a'
    'gucp': 'a', 'gdcp': 'a', # SU/SD shift drains [128,512]
    'sq': 'g',                # squares of gxyb (TT mult, 2048): 'v' or 'g'
    'ss': 'gv',                # sqx+sqy add (1024): 'v' or 'g'
    'acc': 'v',               # channel accumulate (1024): 'v' or 'g'
    'sums': 'v',              # channel sums of gxyb (2048): 'v' or 'g'
    'gg2': 'v',               # square of gsB (2048): 'v' or 'g'
    'thtl': 'v',              # TS scalings of gxs^2 (DVE; GpSimd TS is ~15us!)
    'c0c2': 'v',              # c0M/c2M u16 compares (DVE only)
    'sxy': 'v',               # sign product (TT mult): 'v' or 'g'
    'csM': 'v',               # sign mask (TS, DVE)
    'm1': 'v', 'msel': 'v', 'm0': 'v', 'm2u': 'v',   # TT max: DVE only
    'thr': 'v',               # STT: DVE only
}


def _build_nc(reps=1, eng=None):
    eng = dict(DEFAULT_ENG, **(eng or {}))
    nc = bass.Bass(trn_type="TRN2")
    x_d = nc.dram_tensor("x", (NCH, SH, W + 4), dt.bfloat16, kind="ExternalInput")
    wb_d = nc.dram_tensor("wb", (P, WBW), dt.bfloat16, kind="ExternalInput")
    out_d = nc.dram_tensor("out", (RPC, W), dt.bfloat16, kind="ExternalOutput")

    def EV(key, sc=0):
        e = eng[key]
        e = e[sc % len(e)]
        assert e in ('v', 'g')
        return nc.gpsimd if e == 'g' else nc.vector

    def drain(key, sc, out_ap, ps_ap):
        e = eng[key]
        e = e[sc % len(e)]
        if e == 'a':
            nc.scalar.activation(out_ap, ps_ap, Act.Copy)
        else:
            nc.vector.tensor_copy(out_ap, ps_ap)

    with TileContext(nc) as tc:
        with tc.tile_pool(name="sb", bufs=2) as pool, \
             tc.tile_pool(name="ps", bufs=2, space="PSUM") as pp:
            wb_sb = pool.tile([P, WBW], dt.bfloat16, tag="wb", bufs=1)
            nc.sync.dma_start(wb_sb[:, :], wb_d[:, :])

            # reps>1 exists only for benchmarking (test.py): a hardware loop
            # repeats the identical full pipeline on-device so one dispatch
            # amortizes the host/tunnel launch overhead over `reps` runs
            _rep = tc.For_i(0, reps, 1, hint_engines=(mybir.EngineType.PE, mybir.EngineType.DVE, mybir.EngineType.Activation, mybir.EngineType.SP), staggered_reset=True) if reps > 1 else None
            if _rep is not None:
                _rep.__enter__()
            class _TS:
                pass

            def make_state(t):
                st = _TS()
                base = BASES[t]
                go = GO_T0 if t == 0 else (GO_T4 if t == NT - 1 else GO_MID)
                st.V121 = wb_sb[:, go:go + P]
                st.NV121 = wb_sb[:, go + P:go + 2 * P]
                st.U = wb_sb[:, go + 2 * P:go + 3 * P]
                st.U2 = wb_sb[:, go + 3 * P:go + 4 * P]
                st.SU = wb_sb[:, go + 4 * P:go + 5 * P]
                st.SD = wb_sb[:, go + 5 * P:go + 6 * P]
                st.xs, st.bls = [], []
                for c in range(NCH):
                    x_sb = pool.tile([P, W + 4], dt.bfloat16, tag=f"x{c}", bufs=1)
                    nc.sync.dma_start(x_sb[:, :], x_d[c, base:base + P, :])
                    st.xs.append(x_sb)
                    bl = pool.tile([P, W + 2], dt.bfloat16, tag=f"blur{c}", bufs=2)
                    nc.vector.memset(bl[:, 0:1], 0.0)
                    nc.vector.memset(bl[:, W + 1:W + 2], 0.0)
                    st.bls.append(bl)
                st.grad = pool.tile([P, W + 2], dt.bfloat16, tag="grad", bufs=2)
                nc.vector.memset(st.grad[:, 0:1], 0.0)
                nc.vector.memset(st.grad[:, W + 1:W + 2], 0.0)
                st.gU = pool.tile([P, W + 2], dt.bfloat16, tag="gU", bufs=2)
                st.gD = pool.tile([P, W + 2], dt.bfloat16, tag="gD", bufs=2)
                st.obF = pool.tile([P, W], dt.bfloat16, tag="obF", bufs=2)
                st.gxybs = [None] * NCH
                st.masks = [None] * NSC
                return st

            def blur_sc(st, sc, c):
                # 5x5 blur for superchunk sc, channel c
                ps = pp.tile([P, SCW], dt.float32, tag="psb", bufs=1)
                for half in range(2):
                    j0 = sc * SCW + half * CW
                    for d in range(5):
                        nc.tensor.matmul(
                            out=ps[:, half * CW:half * CW + CW],
                            lhsT=wb_sb[:, d * P:(d + 1) * P],
                            rhs=st.xs[c][:, j0 + d:j0 + d + CW],
                            start=(d == 0), stop=(d == 4),
                        )
                drain('blurcp', sc, st.bls[c][:, 1 + sc * SCW:1 + (sc + 1) * SCW], ps[:, :])

            def stage_b_ch(st, sc, c):
                # sobel + magnitude for superchunk sc, channel c
                bl = st.bls[c]
                pgxy = pp.tile([P, 2 * SCW], dt.float32, tag="pgxy", bufs=1)
                for half in range(2):
                    pj = 1 + sc * SCW + half * CW
                    hx = slice(half * CW, half * CW + CW)
                    hy = slice(SCW + half * CW, SCW + half * CW + CW)
                    nc.tensor.matmul(out=pgxy[:, hx], lhsT=st.V121,
                                     rhs=bl[:, pj - 1:pj - 1 + CW], start=True, stop=False)
                    nc.tensor.matmul(out=pgxy[:, hx], lhsT=st.NV121,
                                     rhs=bl[:, pj + 1:pj + 1 + CW], start=False, stop=True)
                    nc.tensor.matmul(out=pgxy[:, hy], lhsT=st.U,
                                     rhs=bl[:, pj - 1:pj - 1 + CW], start=True, stop=False)
                    nc.tensor.matmul(out=pgxy[:, hy], lhsT=st.U2,
                                     rhs=bl[:, pj:pj + CW], start=False, stop=False)
                    nc.tensor.matmul(out=pgxy[:, hy], lhsT=st.U,
                                     rhs=bl[:, pj + 1:pj + 1 + CW], start=False, stop=True)
                gxyb = pool.tile([P, 2 * SCW], dt.bfloat16, tag="gxyb", bufs=3)
                drain('gxycp', sc, gxyb[:, :], pgxy[:, :])
                st.gxybs[c] = gxyb
                sqf = pool.tile([P, 2 * SCW], dt.bfloat16, tag="sqf", bufs=2)
                sqe = eng['sq'][sc % len(eng['sq'])]
                if sqe == 'a':
                    # square straight off PSUM on ACT (second reader of pgxy)
                    nc.scalar.activation(sqf[:, :], pgxy[:, :], Act.Square)
                else:
                    ev = nc.gpsimd if sqe == 'g' else nc.vector
                    ev.tensor_tensor(sqf[:, 0:SCW], gxyb[:, 0:SCW],
                                     gxyb[:, 0:SCW], Alu.mult)
                    ev.tensor_tensor(sqf[:, SCW:2 * SCW], gxyb[:, SCW:2 * SCW],
                                     gxyb[:, SCW:2 * SCW], Alu.mult)
                # ss = sqx + sqy, in place over the sqx half
                EV('ss', sc).tensor_tensor(sqf[:, 0:SCW], sqf[:, 0:SCW],
                                           sqf[:, SCW:2 * SCW], Alu.add)
                pj = 1 + sc * SCW
                if c == 0:
                    nc.scalar.activation(st.grad[:, pj:pj + SCW], sqf[:, 0:SCW], Act.Sqrt)
                else:
                    # magnitude into the dead sqy half, then accumulate
                    nc.scalar.activation(sqf[:, SCW:2 * SCW], sqf[:, 0:SCW], Act.Sqrt)
                    EV('acc', sc).tensor_tensor(st.grad[:, pj:pj + SCW], st.grad[:, pj:pj + SCW],
                                                sqf[:, SCW:2 * SCW], Alu.add)

            def shifts(st, sc, half):
                # rolling production of the +-1 row shifts of grad via
                # SU/SD band matmuls; window [g0+2, g0+1025] in padded
                # cols (plus a 2-col top-up at the tile start) so that
                # stage_c(sc) reads cols [g0, g0+1025] fully covered
                pU = pp.tile([P, CW], dt.float32, tag="pU", bufs=1)
                pD = pp.tile([P, CW], dt.float32, tag="pD", bufs=1)
                c0 = sc * SCW + 2 + half * CW
                nc.tensor.matmul(out=pU[:, :], lhsT=st.SU, rhs=st.grad[:, c0:c0 + CW],
                                 start=True, stop=True)
                nc.tensor.matmul(out=pD[:, :], lhsT=st.SD, rhs=st.grad[:, c0:c0 + CW],
                                 start=True, stop=True)
                drain('gucp', sc, st.gU[:, c0:c0 + CW], pU[:, :])
                drain('gdcp', sc, st.gD[:, c0:c0 + CW], pD[:, :])
                if sc == 0 and half == 0:
                    pU2 = pp.tile([P, CW], dt.float32, tag="pU", bufs=1)
                    pD2 = pp.tile([P, CW], dt.float32, tag="pD", bufs=1)
                    nc.tensor.matmul(out=pU2[:, 0:2], lhsT=st.SU, rhs=st.grad[:, 0:2],
                                     start=True, stop=True)
                    nc.tensor.matmul(out=pD2[:, 0:2], lhsT=st.SD, rhs=st.grad[:, 0:2],
                                     start=True, stop=True)
                    drain('gucp', sc, st.gU[:, 0:2], pU2[:, 0:2])
                    drain('gdcp', sc, st.gD[:, 0:2], pD2[:, 0:2])

            def masks_sc(st, sc):
                # channel sums + orientation masks for superchunk sc
                gsB = pool.tile([P, 2 * SCW], dt.bfloat16, tag="gsB", bufs=2)
                for hs in (slice(0, SCW), slice(SCW, 2 * SCW)):
                    EV('sums', sc).tensor_tensor(gsB[:, hs], st.gxybs[0][:, hs],
                                                 st.gxybs[1][:, hs], Alu.add)
                    EV('sums', sc).tensor_tensor(gsB[:, hs], gsB[:, hs],
                                                 st.gxybs[2][:, hs], Alu.add)
                sxy = pool.tile([P, SCW], dt.bfloat16, tag="sxy", bufs=2)
                EV('sxy', sc).tensor_tensor(sxy[:, :], gsB[:, 0:SCW], gsB[:, SCW:2 * SCW],
                                            Alu.mult)
                csM = pool.tile([P, SCW], dt.uint16, tag="csM", bufs=2)
                nc.vector.tensor_scalar(csM[:, :], sxy[:, :], 0.0, None, Alu.is_gt)
                # square the sums in place: gsB -> [gxs^2 | gys^2]
                EV('gg2', sc).tensor_tensor(gsB[:, 0:SCW], gsB[:, 0:SCW],
                                            gsB[:, 0:SCW], Alu.mult)
                EV('gg2', sc).tensor_tensor(gsB[:, SCW:2 * SCW], gsB[:, SCW:2 * SCW],
                                            gsB[:, SCW:2 * SCW], Alu.mult)
                # scaled copies of gxs^2 via cheap 4x-mode TS, then u16
                # compares (cheaper than two 1x-mode STT ops)
                th = pool.tile([P, SCW], dt.bfloat16, tag="th", bufs=2)
                EV('thtl', sc).tensor_scalar(th[:, :], gsB[:, 0:SCW], TAN_HI2, None, Alu.mult)
                c2M = pool.tile([P, SCW], dt.uint16, tag="c2M", bufs=2)
                EV('c0c2', sc).tensor_tensor(c2M[:, :], th[:, :],
                                             gsB[:, SCW:2 * SCW], Alu.is_lt)
                tl = pool.tile([P, SCW], dt.bfloat16, tag="th", bufs=2)
                EV('thtl', sc).tensor_scalar(tl[:, :], gsB[:, 0:SCW], TAN_LO2, None, Alu.mult)
                c0M = pool.tile([P, SCW], dt.uint16, tag="c0M", bufs=2)
                EV('c0c2', sc).tensor_tensor(c0M[:, :], tl[:, :],
                                             gsB[:, SCW:2 * SCW], Alu.is_gt)
                return (csM, c0M, c2M)

            def stage_c(st, sc, masks):
                csM, c0M, c2M = masks
                g0 = sc * SCW
                pj = 1 + g0
                m1 = pool.tile([P, SCW], dt.bfloat16, tag="m1", bufs=2)
                EV('m1', sc).tensor_tensor(m1[:, :], st.gD[:, pj + 1:pj + 1 + SCW],
                                           st.gU[:, pj - 1:pj - 1 + SCW], Alu.max)
                msel = pool.tile([P, SCW], dt.bfloat16, tag="msel", bufs=2)
                EV('msel', sc).tensor_tensor(msel[:, :], st.gD[:, pj - 1:pj - 1 + SCW],
                                             st.gU[:, pj + 1:pj + 1 + SCW], Alu.max)
                m0 = pool.tile([P, SCW], dt.bfloat16, tag="m0", bufs=2)
                EV('m0', sc).tensor_tensor(m0[:, :], st.grad[:, pj - 1:pj - 1 + SCW],
                                           st.grad[:, pj + 1:pj + 1 + SCW], Alu.max)
                m2u = pool.tile([P, SCW], dt.bfloat16, tag="m2u", bufs=2)
                EV('m2u', sc).tensor_tensor(m2u[:, :], st.gU[:, pj:pj + SCW],
                                            st.gD[:, pj:pj + SCW], Alu.max)
                nc.vector.copy_predicated(msel[:, :], csM[:, :], m1[:, :])
                nc.vector.copy_predicated(msel[:, :], c0M[:, :], m0[:, :])
                nc.vector.copy_predicated(msel[:, :], c2M[:, :], m2u[:, :])
                # is_max & grad > 6:  max(msel, 6) < grad; the 6-clamp as
                # a cheap in-place 4x-mode TS, then the compare (reuse m0)
                nc.vector.tensor_scalar(msel[:, :], msel[:, :], LOWER_T, None, Alu.max)
                EV('thr', sc).tensor_tensor(m0[:, :], msel[:, :],
                                            st.grad[:, pj:pj + SCW], Alu.is_lt)
                # (grad <= 50) * ig
                EV('thr', sc).scalar_tensor_tensor(st.obF[:, g0:g0 + SCW], st.grad[:, pj:pj + SCW],
                                                   UPPER_T, m0[:, :], Alu.is_le, Alu.mult)

            # flat cross-tile pipeline: position g covers blur(g),
            # stage_b(g-1), shifts/stage_c(g-2); tile t+1's state (x DMAs,
            # memsets) is created one position before its first blur so the
            # input loads hide behind the previous tile's tail
            TOT = NT * NSC
            states = {0: make_state(0)}
            for g in range(TOT + 2):
                tb, scb = divmod(g, NSC)
                ts_, scs = divmod(g - 1, NSC)
                tc_, scc = divmod(g - 2, NSC)
                if scb == NSC - 1 and tb + 1 < NT:
                    states[tb + 1] = make_state(tb + 1)
                for c in range(NCH):
                    if g < TOT:
                        blur_sc(states[tb], scb, c)
                    if 0 <= g - 1 < TOT:
                        stage_b_ch(states[ts_], scs, c)
                    if 0 <= g - 2 < TOT and c < 2:
                        shifts(states[tc_], scc, c)
                if 0 <= g - 1 < TOT:
                    states[ts_].masks[scs] = masks_sc(states[ts_], scs)
                if 0 <= g - 2 < TOT:
                    stage_c(states[tc_], scc, states[tc_].masks[scc])
                    states[tc_].masks[scc] = None
                    if scc == NSC - 1:
                        st = states.pop(tc_)
                        if tc_ < NT - 1:
                            nc.sync.dma_start(out_d[tc_ * 120:tc_ * 120 + 120, :], st.obF[4:124, :])
                        else:
                            nc.sync.dma_start(out_d[480:512, :], st.obF[92:124, :])

            if _rep is not None:
                _rep.__exit__(None, None, None)

    import bass_rust
    # HW descriptors hold only one sync wait; park extras on Ldweights /
    # split the remainder into EventSemaphore instructions
    bass_rust.move_matmul_waits_to_ldweights(nc.m)
    bass_rust.generate_event_semaphores(nc)
    nc.finalize()
    return nc


def _shard_inputs(img, gauss):
    imgf = np.ascontiguousarray(img[0])  # [3, H, W] f32
    in_maps = []
    for k in range(NCORES):
        xk = np.zeros((NCH, SH, W + 4), dtype=BF16)
        lo = k * RPC - HALO
        hi = k * RPC + RPC + HALO
        s0, s1 = max(lo, 0), min(hi, H)
        xk[:, s0 - lo:s1 - lo, 2:W + 2] = imgf[:, s0:s1, :].astype(BF16)
        wbk = _weights(gauss, is_top=(k == 0), is_bot=(k == NCORES - 1))
        in_maps.append({"x": xk, "wb": wbk})
    return in_maps


def _assemble(results):
    full = np.empty((H, W), dtype=np.float32)
    for k in range(NCORES):
        ok = np.asarray(results[k]["out"])  # [512, 4096] bf16 0/1
        full[k * RPC:(k + 1) * RPC] = (ok != 0).astype(np.float32)
    return full.reshape(1, 1, H, W)


def _run(img, gauss, trace=False):
    nc = _build_nc()
    in_maps = _shard_inputs(np.asarray(img, np.float32), np.asarray(gauss, np.float32))
    res = run_bass_kernel_spmd(nc, in_maps, core_ids=list(range(NCORES)), trace=trace)
    return _assemble(res.results), res.exec_time_ns


def kernel(img=None, gauss=None, sobel=None, dir_w=None, **_):
    out, _ns = _run(img, gauss)
    return out


# revision 32
# speedup vs baseline: 1.3741x; 1.0797x over previous
import sys

sys.path.insert(0, "/opt/trn_rl_repo")

import numpy as np
import ml_dtypes

from concourse import bass, mybir
from concourse.tile import TileContext
from concourse.bass_utils import run_bass_kernel_spmd

dt = mybir.dt
Alu = mybir.AluOpType
Act = mybir.ActivationFunctionType

H = 4096
W = 4096
NCORES = 8
RPC = H // NCORES            # 512 output rows per core
HALO = 4                     # blur(2) + sobel(1) + nms(1)
SH = RPC + 2 * HALO          # 520 input rows per core
BASES = (0, 120, 240, 360, 392)
NT = len(BASES)
NCH = 3
CW = 512                     # matmul chunk (one PSUM bank)
SCW = 1024                   # superchunk for elementwise stages
NSC = W // SCW
P = 128
BF16 = ml_dtypes.bfloat16

TAN_LO = float(np.float32(np.tan(3.14159 / 8)))
TAN_HI = float(np.float32(np.tan(3 * 3.14159 / 8)))
TAN_LO2 = TAN_LO * TAN_LO
TAN_HI2 = TAN_HI * TAN_HI
LOWER_T = 6.0
UPPER_T = 50.0

# wb column layout: 5 blur bands, then per tile-variant {mid, t0, t4} the
# six sobel/shift bands [V121, NV121, U, U2, SU, SD]
GO_MID = 5 * P
GO_T0 = GO_MID + 6 * P
GO_T4 = GO_T0 + 6 * P
WBW = GO_T4 + 6 * P          # 2944


def _band(taps, r):
    # lhsT[k, m] = taps[k - m + r]  => out[m] = sum_k taps[k-m+r] * x[k]
    L = np.zeros((P, P), np.float32)
    for i, tv in enumerate(taps):
        L += np.float32(tv) * np.eye(P, k=r - i, dtype=np.float32)
    return L


def _weights(gauss, is_top, is_bot):
    g = np.asarray(gauss, np.float32)
    bg = _band(g, 2)
    v121 = _band([1.0, 2.0, 1.0], 1)
    u = _band([1.0, 0.0, -1.0], 1)
    su = _band([1.0], 1)
    sd = _band([1.0], -1)

    def group(zero_row, zero_su, zero_sd):
        mats = [v121.copy(), -v121, u.copy(), 2.0 * u, su.copy(), sd.copy()]
        if zero_row is not None:
            for idx in (0, 1, 2, 3):
                mats[idx][zero_row, :] = 0.0
            if zero_su:
                mats[4][zero_row, :] = 0.0
            if zero_sd:
                mats[5][zero_row, :] = 0.0
        return mats

    cols = [bg * g[d] for d in range(5)]
    cols += group(None, False, False)                       # mid
    cols += group(3 if is_top else None, True, False)       # t0 variant
    cols += group(124 if is_bot else None, False, True)     # t4 variant
    wb = np.concatenate(cols, axis=1)
    assert wb.shape == (P, WBW)
    return wb.astype(BF16)


# per-site engine assignment ('v'=DVE, 'a'=ACT for PSUM drains; 'g'=GpSimd is
# only legal for plain TT add/sub/mult + tensor_scalar, and on real HW costs
# ~2.6us per 1024-wide op (~3x DVE), so use it sparingly).
# Multi-char values alternate by superchunk index for fractional balance.
DEFAULT_ENG = {
    'blurcp': 'a',            # blur PSUM drain [128,1024]: 'v' or 'a'
    'gxycp': 'a',             # fused gx|gy PSUM drain [128,2048]: 'v' or 'a'
    'gucp': 'a', 'gdcp': 'a', # SU/SD shift drains [128,512]
    'sq': 'g',                # squares of gxyb (TT mult, 2048): 'v' or 'g'
    'ss': 'gv',                # sqx+sqy add (1024): 'v' or 'g'
    'acc': 'v',               # channel accumulate (1024): 'v' or 'g'
    'sums': 'v',              # channel sums of gxyb (2048): 'v' or 'g'
    'gg2': 'v',               # square of gsB (2048): 'v' or 'g'
    'thtl': 'v',              # TS scalings of gxs^2 (DVE; GpSimd TS is ~15us!)
    'c0c2': 'v',              # c0M/c2M u16 compares (DVE only)
    'sxy': 'v',               # sign product (TT mult): 'v' or 'g'
    'csM': 'v',               # sign mask (TS, DVE)
    'm1': 'v', 'msel': 'v', 'm0': 'v', 'm2u': 'v',   # TT max: DVE only
    'thr': 'v',               # STT: DVE only
}


def _build_nc(reps=1, eng=None):
    eng = dict(DEFAULT_ENG, **(eng or {}))
    nc = bass.Bass(trn_type="TRN2")
    x_d = nc.dram_tensor("x", (NCH, SH, W + 4), dt.bfloat16, kind="ExternalInput")
    wb_d = nc.dram_tensor("wb", (P, WBW), dt.bfloat16, kind="ExternalInput")
    out_d = nc.dram_tensor("out", (RPC, W), dt.bfloat16, kind="ExternalOutput")

    def EV(key, sc=0):
        e = eng[key]
        e = e[sc % len(e)]
        assert e in ('v', 'g')
        return nc.gpsimd if e == 'g' else nc.vector

    def drain(key, sc, out_ap, ps_ap):
        e = eng[key]
        e = e[sc % len(e)]
        if e == 'a':
            nc.scalar.activation(out_ap, ps_ap, Act.Copy)
        else:
            nc.vector.tensor_copy(out_ap, ps_ap)

    with TileContext(nc) as tc:
        with tc.tile_pool(name="sb", bufs=2) as pool, \
             tc.tile_pool(name="ps", bufs=2, space="PSUM") as pp:
            wb_sb = pool.tile([P, WBW], dt.bfloat16, tag="wb", bufs=1)
            nc.sync.dma_start(wb_sb[:, :], wb_d[:, :])

            # reps>1 exists only for benchmarking (test.py): a hardware loop
            # repeats the identical full pipeline on-device so one dispatch
            # amortizes the host/tunnel launch overhead over `reps` runs
            _rep = tc.For_i(0, reps, 1, hint_engines=(mybir.EngineType.PE, mybir.EngineType.DVE, mybir.EngineType.Activation, mybir.EngineType.SP), staggered_reset=True) if reps > 1 else None
            if _rep is not None:
                _rep.__enter__()
            class _TS:
                pass

            def make_state(t):
                st = _TS()
                base = BASES[t]
                go = GO_T0 if t == 0 else (GO_T4 if t == NT - 1 else GO_MID)
                st.V121 = wb_sb[:, go:go + P]
                st.NV121 = wb_sb[:, go + P:go + 2 * P]
                st.U = wb_sb[:, go + 2 * P:go + 3 * P]
                st.U2 = wb_sb[:, go + 3 * P:go + 4 * P]
                st.SU = wb_sb[:, go + 4 * P:go + 5 * P]
                st.SD = wb_sb[:, go + 5 * P:go + 6 * P]
                st.xs, st.bls = [], []
                for c in range(NCH):
                    x_sb = pool.tile([P, W + 4], dt.bfloat16, tag=f"x{c}", bufs=1)
                    nc.sync.dma_start(x_sb[:, :], x_d[c, base:base + P, :])
                    st.xs.append(x_sb)
                    bl = pool.tile([P, W + 2], dt.bfloat16, tag=f"blur{c}", bufs=2)
                    nc.vector.memset(bl[:, 0:1], 0.0)
                    nc.vector.memset(bl[:, W + 1:W + 2], 0.0)
                    st.bls.append(bl)
                st.grad = pool.tile([P, W + 2], dt.bfloat16, tag="grad", bufs=2)
                nc.vector.memset(st.grad[:, 0:1], 0.0)
                nc.vector.memset(st.grad[:, W + 1:W + 2], 0.0)
                st.gU = pool.tile([P, W + 2], dt.bfloat16, tag="gU", bufs=2)
                st.gD = pool.tile([P, W + 2], dt.bfloat16, tag="gD", bufs=2)
                st.obF = pool.tile([P, W], dt.bfloat16, tag="obF", bufs=1)
                st.blS = pool.tile([P, W + 2], dt.bfloat16, tag="blS", bufs=1)
                nc.vector.memset(st.blS[:, 0:1], 0.0)
                nc.vector.memset(st.blS[:, W + 1:W + 2], 0.0)
                st.masks = [None] * NSC
                return st

            def blur_sc(st, sc, c):
                # 5x5 blur for superchunk sc, channel c
                ps = pp.tile([P, SCW], dt.float32, tag="psb", bufs=1)
                for half in range(2):
                    j0 = sc * SCW + half * CW
                    for d in range(5):
                        nc.tensor.matmul(
                            out=ps[:, half * CW:half * CW + CW],
                            lhsT=wb_sb[:, d * P:(d + 1) * P],
                            rhs=st.xs[c][:, j0 + d:j0 + d + CW],
                            start=(d == 0), stop=(d == 4),
                        )
                drain('blurcp', sc, st.bls[c][:, 1 + sc * SCW:1 + (sc + 1) * SCW], ps[:, :])

            def stage_b_ch(st, sc, c):
                # sobel + magnitude for superchunk sc, channel c
                bl = st.bls[c]
                pgxy = pp.tile([P, 2 * SCW], dt.float32, tag="pgxy", bufs=1)
                for half in range(2):
                    pj = 1 + sc * SCW + half * CW
                    hx = slice(half * CW, half * CW + CW)
                    hy = slice(SCW + half * CW, SCW + half * CW + CW)
                    nc.tensor.matmul(out=pgxy[:, hx], lhsT=st.V121,
                                     rhs=bl[:, pj - 1:pj - 1 + CW], start=True, stop=False)
                    nc.tensor.matmul(out=pgxy[:, hx], lhsT=st.NV121,
                                     rhs=bl[:, pj + 1:pj + 1 + CW], start=False, stop=True)
                    nc.tensor.matmul(out=pgxy[:, hy], lhsT=st.U,
                                     rhs=bl[:, pj - 1:pj - 1 + CW], start=True, stop=False)
                    nc.tensor.matmul(out=pgxy[:, hy], lhsT=st.U2,
                                     rhs=bl[:, pj:pj + CW], start=False, stop=False)
                    nc.tensor.matmul(out=pgxy[:, hy], lhsT=st.U,
                                     rhs=bl[:, pj + 1:pj + 1 + CW], start=False, stop=True)
                # the signed per-channel gx/gy are never needed: square
                # straight off PSUM on ACT (sole reader of pgxy); channel
                # sums come from the summed-blur matmul path in masks_sc
                sqf = pool.tile([P, 2 * SCW], dt.bfloat16, tag="sqf", bufs=2)
                nc.scalar.activation(sqf[:, :], pgxy[:, :], Act.Square)
                # ss = sqx + sqy, in place over the sqx half
                EV('ss', sc).tensor_tensor(sqf[:, 0:SCW], sqf[:, 0:SCW],
                                           sqf[:, SCW:2 * SCW], Alu.add)
                pj = 1 + sc * SCW
                if c == 0:
                    nc.scalar.activation(st.grad[:, pj:pj + SCW], sqf[:, 0:SCW], Act.Sqrt)
                else:
                    # magnitude into the dead sqy half, then accumulate
                    nc.scalar.activation(sqf[:, SCW:2 * SCW], sqf[:, 0:SCW], Act.Sqrt)
                    EV('acc', sc).tensor_tensor(st.grad[:, pj:pj + SCW], st.grad[:, pj:pj + SCW],
                                                sqf[:, SCW:2 * SCW], Alu.add)

            def shifts(st, sc, half):
                # rolling production of the +-1 row shifts of grad via
                # SU/SD band matmuls; window [g0+2, g0+1025] in padded
                # cols (plus a 2-col top-up at the tile start) so that
                # stage_c(sc) reads cols [g0, g0+1025] fully covered
                pU = pp.tile([P, CW], dt.float32, tag="pU", bufs=1)
                pD = pp.tile([P, CW], dt.float32, tag="pD", bufs=1)
                c0 = sc * SCW + 2 + half * CW
                nc.tensor.matmul(out=pU[:, :], lhsT=st.SU, rhs=st.grad[:, c0:c0 + CW],
                                 start=True, stop=True)
                nc.tensor.matmul(out=pD[:, :], lhsT=st.SD, rhs=st.grad[:, c0:c0 + CW],
                                 start=True, stop=True)
                drain('gucp', sc, st.gU[:, c0:c0 + CW], pU[:, :])
                drain('gdcp', sc, st.gD[:, c0:c0 + CW], pD[:, :])
                if sc == 0 and half == 0:
                    pU2 = pp.tile([P, CW], dt.float32, tag="pU", bufs=1)
                    pD2 = pp.tile([P, CW], dt.float32, tag="pD", bufs=1)
                    nc.tensor.matmul(out=pU2[:, 0:2], lhsT=st.SU, rhs=st.grad[:, 0:2],
                                     start=True, stop=True)
                    nc.tensor.matmul(out=pD2[:, 0:2], lhsT=st.SD, rhs=st.grad[:, 0:2],
                                     start=True, stop=True)
                    drain('gucp', sc, st.gU[:, 0:2], pU2[:, 0:2])
                    drain('gdcp', sc, st.gD[:, 0:2], pD2[:, 0:2])

            def masks_sc(st, sc):
                # channel sums via conv linearity: sum the blurred channels
                # (window covers this superchunk +1 col each side; the last
                # superchunk stops at the zero pad col), then one extra
                # gx|gy matmul set on the sum, drained to gsB
                g0w = sc * SCW
                wend = min(1 + g0w + SCW + 2, W + 1)
                EV('sums', sc).tensor_tensor(st.blS[:, 1 + g0w:wend],
                                             st.bls[0][:, 1 + g0w:wend],
                                             st.bls[1][:, 1 + g0w:wend], Alu.add)
                EV('sums', sc).tensor_tensor(st.blS[:, 1 + g0w:wend],
                                             st.blS[:, 1 + g0w:wend],
                                             st.bls[2][:, 1 + g0w:wend], Alu.add)
                psS = pp.tile([P, 2 * SCW], dt.float32, tag="pgxy", bufs=1)
                for half in range(2):
                    pj = 1 + sc * SCW + half * CW
                    hx = slice(half * CW, half * CW + CW)
                    hy = slice(SCW + half * CW, SCW + half * CW + CW)
                    nc.tensor.matmul(out=psS[:, hx], lhsT=st.V121,
                                     rhs=st.blS[:, pj - 1:pj - 1 + CW], start=True, stop=False)
                    nc.tensor.matmul(out=psS[:, hx], lhsT=st.NV121,
                                     rhs=st.blS[:, pj + 1:pj + 1 + CW], start=False, stop=True)
                    nc.tensor.matmul(out=psS[:, hy], lhsT=st.U,
                                     rhs=st.blS[:, pj - 1:pj - 1 + CW], start=True, stop=False)
                    nc.tensor.matmul(out=psS[:, hy], lhsT=st.U2,
                                     rhs=st.blS[:, pj:pj + CW], start=False, stop=False)
                    nc.tensor.matmul(out=psS[:, hy], lhsT=st.U,
                                     rhs=st.blS[:, pj + 1:pj + 1 + CW], start=False, stop=True)
                gsB = pool.tile([P, 2 * SCW], dt.bfloat16, tag="gsB", bufs=2)
                drain('sumcp', sc, gsB[:, :], psS[:, :])
                sxy = pool.tile([P, SCW], dt.bfloat16, tag="sxy", bufs=2)
                EV('sxy', sc).tensor_tensor(sxy[:, :], gsB[:, 0:SCW], gsB[:, SCW:2 * SCW],
                                            Alu.mult)
                csM = pool.tile([P, SCW], dt.uint16, tag="csM", bufs=2)
                nc.vector.tensor_scalar(csM[:, :], sxy[:, :], 0.0, None, Alu.is_gt)
                # square the sums in place: gsB -> [gxs^2 | gys^2]
                EV('gg2', sc).tensor_tensor(gsB[:, 0:SCW], gsB[:, 0:SCW],
                                            gsB[:, 0:SCW], Alu.mult)
                EV('gg2', sc).tensor_tensor(gsB[:, SCW:2 * SCW], gsB[:, SCW:2 * SCW],
                                            gsB[:, SCW:2 * SCW], Alu.mult)
                # scaled copies of gxs^2 via cheap 4x-mode TS, then u16
                # compares (cheaper than two 1x-mode STT ops)
                th = pool.tile([P, SCW], dt.bfloat16, tag="th", bufs=2)
                EV('thtl', sc).tensor_scalar(th[:, :], gsB[:, 0:SCW], TAN_HI2, None, Alu.mult)
                c2M = pool.tile([P, SCW], dt.uint16, tag="c2M", bufs=2)
                EV('c0c2', sc).tensor_tensor(c2M[:, :], th[:, :],
                                             gsB[:, SCW:2 * SCW], Alu.is_lt)
                tl = pool.tile([P, SCW], dt.bfloat16, tag="th", bufs=2)
                EV('thtl', sc).tensor_scalar(tl[:, :], gsB[:, 0:SCW], TAN_LO2, None, Alu.mult)
                c0M = pool.tile([P, SCW], dt.uint16, tag="c0M", bufs=2)
                EV('c0c2', sc).tensor_tensor(c0M[:, :], tl[:, :],
                                             gsB[:, SCW:2 * SCW], Alu.is_gt)
                return (csM, c0M, c2M)

            def stage_c(st, sc, masks):
                csM, c0M, c2M = masks
                g0 = sc * SCW
                pj = 1 + g0
                m1 = pool.tile([P, SCW], dt.bfloat16, tag="m1", bufs=2)
                EV('m1', sc).tensor_tensor(m1[:, :], st.gD[:, pj + 1:pj + 1 + SCW],
                                           st.gU[:, pj - 1:pj - 1 + SCW], Alu.max)
                msel = pool.tile([P, SCW], dt.bfloat16, tag="msel", bufs=2)
                EV('msel', sc).tensor_tensor(msel[:, :], st.gD[:, pj - 1:pj - 1 + SCW],
                                             st.gU[:, pj + 1:pj + 1 + SCW], Alu.max)
                m0 = pool.tile([P, SCW], dt.bfloat16, tag="m0", bufs=2)
                EV('m0', sc).tensor_tensor(m0[:, :], st.grad[:, pj - 1:pj - 1 + SCW],
                                           st.grad[:, pj + 1:pj + 1 + SCW], Alu.max)
                m2u = pool.tile([P, SCW], dt.bfloat16, tag="m2u", bufs=2)
                EV('m2u', sc).tensor_tensor(m2u[:, :], st.gU[:, pj:pj + SCW],
                                            st.gD[:, pj:pj + SCW], Alu.max)
                nc.vector.copy_predicated(msel[:, :], csM[:, :], m1[:, :])
                nc.vector.copy_predicated(msel[:, :], c0M[:, :], m0[:, :])
                nc.vector.copy_predicated(msel[:, :], c2M[:, :], m2u[:, :])
                # is_max & grad > 6:  max(msel, 6) < grad; the 6-clamp as
                # a cheap in-place 4x-mode TS, then the compare (reuse m0)
                nc.vector.tensor_scalar(msel[:, :], msel[:, :], LOWER_T, None, Alu.max)
                EV('thr', sc).tensor_tensor(m0[:, :], msel[:, :],
                                            st.grad[:, pj:pj + SCW], Alu.is_lt)
                # (grad <= 50) * ig
                EV('thr', sc).scalar_tensor_tensor(st.obF[:, g0:g0 + SCW], st.grad[:, pj:pj + SCW],
                                                   UPPER_T, m0[:, :], Alu.is_le, Alu.mult)

            # flat cross-tile pipeline: position g covers blur(g),
            # stage_b(g-1), shifts/stage_c(g-2); tile t+1's state (x DMAs,
            # memsets) is created one position before its first blur so the
            # input loads hide behind the previous tile's tail
            TOT = NT * NSC
            states = {0: make_state(0)}
            for g in range(TOT + 2):
                tb, scb = divmod(g, NSC)
                ts_, scs = divmod(g - 1, NSC)
                tc_, scc = divmod(g - 2, NSC)
                if scb == NSC - 1 and tb + 1 < NT:
                    states[tb + 1] = make_state(tb + 1)
                for c in range(NCH):
                    if g < TOT:
                        blur_sc(states[tb], scb, c)
                    if 0 <= g - 1 < TOT:
                        stage_b_ch(states[ts_], scs, c)
                    if 0 <= g - 2 < TOT and c < 2:
                        shifts(states[tc_], scc, c)
                if 0 <= g - 1 < TOT:
                    states[ts_].masks[scs] = masks_sc(states[ts_], scs)
                if 0 <= g - 2 < TOT:
                    stage_c(states[tc_], scc, states[tc_].masks[scc])
                    states[tc_].masks[scc] = None
                    if scc == NSC - 1:
                        st = states.pop(tc_)
                        if tc_ < NT - 1:
                            nc.sync.dma_start(out_d[tc_ * 120:tc_ * 120 + 120, :], st.obF[4:124, :])
                        else:
                            nc.sync.dma_start(out_d[480:512, :], st.obF[92:124, :])

            if _rep is not None:
                _rep.__exit__(None, None, None)

    import bass_rust
    # HW descriptors hold only one sync wait; park extras on Ldweights /
    # split the remainder into EventSemaphore instructions
    bass_rust.move_matmul_waits_to_ldweights(nc.m)
    bass_rust.generate_event_semaphores(nc)
    nc.finalize()
    return nc


def _shard_inputs(img, gauss):
    imgf = np.ascontiguousarray(img[0])  # [3, H, W] f32
    in_maps = []
    for k in range(NCORES):
        xk = np.zeros((NCH, SH, W + 4), dtype=BF16)
        lo = k * RPC - HALO
        hi = k * RPC + RPC + HALO
        s0, s1 = max(lo, 0), min(hi, H)
        xk[:, s0 - lo:s1 - lo, 2:W + 2] = imgf[:, s0:s1, :].astype(BF16)
        wbk = _weights(gauss, is_top=(k == 0), is_bot=(k == NCORES - 1))
        in_maps.append({"x": xk, "wb": wbk})
    return in_maps


def _assemble(results):
    full = np.empty((H, W), dtype=np.float32)
    for k in range(NCORES):
        ok = np.asarray(results[k]["out"])  # [512, 4096] bf16 0/1
        full[k * RPC:(k + 1) * RPC] = (ok != 0).astype(np.float32)
    return full.reshape(1, 1, H, W)


def _run(img, gauss, trace=False):
    nc = _build_nc()
    in_maps = _shard_inputs(np.asarray(img, np.float32), np.asarray(gauss, np.float32))
    res = run_bass_kernel_spmd(nc, in_maps, core_ids=list(range(NCORES)), trace=trace)
    return _assemble(res.results), res.exec_time_ns


def kernel(img=None, gauss=None, sobel=None, dir_w=None, **_):
    out, _ns = _run(img, gauss)
    return out
